# revision 22
# baseline (speedup 1.0000x reference)
"""BitFeedForward TRN2 kernel: 8-way data-parallel over tokens.

Math (value-equivalent to reference):
  bitlinear(x, w, b) = act_quant(rmsnorm(x)) @ weight_quant(w).T + b
  -> n = round(x * 127/max|x|)          (exact ints, bf16)
     t = clip(round(w/mean|w|), -1, 1)  (ternary, fp8e4 - exact)
     h = (n @ t.T) * alpha + b,  alpha = mean|w| * rsqrt(mean(x^2)+eps) * max|x| / 127
  out = bitlinear(gelu(bitlinear(x,w1,b1)), w2, b2)

Structure per core (x [2048,1024]):
  phase A (per 128-token tile): quantize x -> n1 -> n1T (dma transpose);
    mm1 (bf16 x fp8, exact ints); gelu+alpha1 on ACT from PSUM; raw-h max off
    PSUM (gelu monotone => amax2 = gelu(al1*max h)); Sum(g^2) via ACT Square
    accum (exact rmsnorm-2 stats); quantize -> n2 (round step 1 on ACT via
    FMA, step 2 on DVE); n2 -> DRAM; alpha2 for this tile computed here (LAG
    ahead of its use) so the mm2 drain never waits on fresh smalls.
  phase B (per tile): transpose-load n2T from DRAM; mm2; scale on ACT; store.
  Weight prep: two-pass stream (pass1 abs-sum split DVE/ACT-Abs-accum, pass2
    ternarize + DVE round, bf16 transpose via DMA xbar, fp8 cast). w2 prep is
    interleaved into phase A iterations, emitted after the mm work of each
    iteration so its bulk ops queue behind the critical chains.
  All rsqrt-like smalls use a DVE bit-hack Newton rsqrt (2 iters, ~4e-6 rel)
    => zero ACT Sqrt calls => the whole kernel stays on the gelu_and_others
    activation table set (no LoadActFuncSet churn).
"""

import sys

sys.path.insert(0, "/opt/trn_rl_repo")

from contextlib import ExitStack

import numpy as np

import concourse.bass as bass
import concourse.tile as tile
from concourse import bacc, bass_utils, mybir
from concourse.masks import make_identity

F32 = mybir.dt.float32
BF16 = mybir.dt.bfloat16
FP8 = mybir.dt.float8e4
U32 = mybir.dt.uint32
AX = mybir.AxisListType
OP = mybir.AluOpType
AF = mybir.ActivationFunctionType

RND = 12582912.0  # 1.5*2^23: +RND,-RND rounds fp32 to nearest int (RNE)
EPS = 1e-5
NCORES = 8


def build_bitffn(T_core=2048, D=1024, F=4096, has_b1=False, has_b2=False,
                 FB=4, XCH=2, WR1=2, LAG_P=16, W2SP=13, W2START=1,
                 REPEAT=1, QSPLIT=1.0, XR_POOL=True,
                 SKIP_B=False, SKIP_W2=False, SKIP_SQ=False,
                 SKIP_GMAX=False, SKIP_RND=False, WABS_DVE=False):
    TT = T_core // 128     # token tiles
    KD = D // 128          # k-subtiles for mm1
    KF = F // 128          # k-subtiles for mm2
    NF1 = F // 512         # mm1 psum chunks
    ND2 = D // 512         # mm2 psum chunks
    NXC = (TT + XCH - 1) // XCH          # x chunks
    NW1C = F // (128 * WR1)              # w1 chunks [128, WR1, D]
    W2CW = 2048 if F % 2048 == 0 else F  # w2 chunk width
    NW2H = F // W2CW
    NW2C = (D // 128) * NW2H             # w2 chunks [128, W2CW]

    nc = bacc.Bacc(
        "TRN2", target_bir_lowering=False, debug=False, enable_asserts=True
    )
    x_d = nc.dram_tensor("x", [T_core, D], F32, kind="ExternalInput").ap()
    w1_d = nc.dram_tensor("w1", [F, D], F32, kind="ExternalInput").ap()
    b1_d = nc.dram_tensor("b1", [1, F], F32, kind="ExternalInput").ap()
    w2_d = nc.dram_tensor("w2", [D, F], F32, kind="ExternalInput").ap()
    b2_d = nc.dram_tensor("b2", [1, D], F32, kind="ExternalInput").ap()
    out_d = nc.dram_tensor("out", [T_core, D], F32, kind="ExternalOutput").ap()
    n2_d = nc.dram_tensor("n2scratch", [T_core, F], BF16, kind="Internal").ap()
    rep_d = (nc.dram_tensor("reptag", [1, REPEAT], F32, kind="ExternalInput").ap()
             if REPEAT > 1 else None)

    with tile.TileContext(nc) as tc, ExitStack() as ctx:
        singles = ctx.enter_context(tc.tile_pool(name="singles", bufs=1))
        xp = ctx.enter_context(tc.tile_pool(name="xp", bufs=2))
        n1p = ctx.enter_context(tc.tile_pool(name="n1p", bufs=1))
        n1tp = ctx.enter_context(tc.tile_pool(name="n1tp", bufs=2 * XCH + 1))
        gp = ctx.enter_context(tc.tile_pool(name="gp", bufs=2))
        n2p = ctx.enter_context(tc.tile_pool(name="n2p", bufs=1))
        n2tp = ctx.enter_context(tc.tile_pool(name="n2tp", bufs=3))
        outp = ctx.enter_context(tc.tile_pool(name="outp", bufs=2))
        scp = ctx.enter_context(tc.tile_pool(name="scp", bufs=3))
        wstage = ctx.enter_context(tc.tile_pool(name="wstage", bufs=2))
        deadp = ctx.enter_context(tc.tile_pool(name="deadp", bufs=1))
        ps1 = ctx.enter_context(tc.tile_pool(name="ps1", bufs=5, space="PSUM"))
        ps2 = ctx.enter_context(tc.tile_pool(name="ps2", bufs=ND2 + 1, space="PSUM"))

        # ---- persistent tiles ----
        w1qT = singles.tile([128, KD, F], FP8)     # 32KB/part
        w2qT = singles.tile([128, KF, D], FP8)     # 32KB/part
        ones_r = singles.tile([1, 128], F32)
        nc.vector.memset(ones_r, 1.0)
        ones_c = singles.tile([128, 1], F32)
        nc.vector.memset(ones_c, 1.0)
        if rep_d is not None:
            rtag = singles.tile([1, REPEAT], F32)
            nc.sync.dma_start(rtag, rep_d)

        if has_b1 or has_b2:
            ident = singles.tile([128, 128], F32)
            make_identity(nc, ident)
        ident_bf = singles.tile([128, 128], BF16, tag="ident_bf")
        make_identity(nc, ident_bf)
        amax1_a = singles.tile([128, TT], F32)
        ssq1_a = singles.tile([128, TT], F32)
        c1_a = singles.tile([128, TT], F32)
        al1_a = singles.tile([128, TT], F32)
        sx_a = singles.tile([128, TT, D // 512], F32)
        gmax_a = singles.tile([128, TT, NF1], F32)
        ssg_a = singles.tile([128, TT, 2], F32)    # Sum(g^2) quarters a
        ssg2_a = singles.tile([128, TT, 2], F32)   # Sum(g^2) quarters b
        if SKIP_SQ:
            nc.gpsimd.memset(ssg_a, 1.0)
            nc.gpsimd.memset(ssg2_a, 1.0)
        amax2_a = singles.tile([128, TT], F32)
        c2_a = singles.tile([128, TT], F32)
        al2_a = singles.tile([128, TT], F32)
        w1part = singles.tile([128, NW1C], F32)
        w2part = singles.tile([128, NW2C], F32)
        # w scale smalls, pre-created so alpha2 can be emitted before the
        # w2 finalize site without a python-ordering problem
        s1_b = singles.tile([128, 1], F32, tag="w1_sb")
        k1_b = singles.tile([128, 1], F32, tag="w1_kb")
        s2_b = singles.tile([128, 1], F32, tag="w2_sb")
        k2_b = singles.tile([128, 1], F32, tag="w2_kb")
        if SKIP_W2:
            nc.gpsimd.memset(s2_b, 1.0)
            nc.gpsimd.memset(k2_b, 1.0)
            nc.gpsimd.memset(w2qT, 0.0)

        dead_g = deadp.tile([128, F // 4], BF16, tag="dead_g")
        magic = singles.tile([128, 2], F32, tag="magic")
        nc.vector.memset(magic, float(np.uint32(0x5F3759DF).view(np.float32)))

        def rsqrt_col(u, tag):
            """Returns y = rsqrt(u) on DVE: bit-hack seed + 2 Newton iters
            (~4e-6 rel err). u > 0 f32 [128, c]."""
            cols = u.shape[-1]
            y = scp.tile([128, cols], F32, tag=f"{tag}_y", bufs=2)
            h = scp.tile([128, cols], F32, tag=f"{tag}_h", bufs=2)
            ui = u.bitcast(U32)
            yi = y.bitcast(U32)
            hi = h.bitcast(U32)
            nc.vector.tensor_scalar(hi, ui, 1, None,
                                    op0=OP.logical_shift_right)
            nc.vector.tensor_tensor(yi, magic[:, :cols].bitcast(U32), hi,
                                    OP.subtract)
            for _ in range(3):
                nc.vector.tensor_tensor(h, u, y, OP.mult)
                nc.vector.tensor_tensor(h, h, y, OP.mult)
                nc.vector.tensor_scalar(h, h, -0.5, 1.5, op0=OP.mult,
                                        op1=OP.add)
                nc.vector.tensor_tensor(y, y, h, OP.mult)
            return y

        def finalize_mean(part_tile, nelem, s_b, k_b):
            """abs-sum partials [128, C] -> s_b = 1/clip(mean,1e-5) bcast,
            k_b = clip(mean)/127 bcast (both [128,1])."""
            rowsum = scp.tile([128, 1], F32, tag="w_rowsum")
            nc.vector.tensor_reduce(rowsum, part_tile, axis=AX.X, op=OP.add)
            tot_ps = ps2.tile([128, 128], F32, tag="ps2")
            nc.tensor.matmul(tot_ps[:1, :1], lhsT=rowsum, rhs=ones_c,
                             start=True, stop=True)
            mw = scp.tile([1, 1], F32, tag="w_mw")
            nc.vector.tensor_scalar(mw, tot_ps[:1, :1], 1.0 / nelem, 1e-5,
                                    op0=OP.mult, op1=OP.max)
            s = scp.tile([1, 1], F32, tag="w_s")
            nc.vector.reciprocal(s, mw)
            k = scp.tile([1, 1], F32, tag="w_k")
            nc.vector.tensor_scalar_mul(k, mw, 1.0 / 127.0)
            for src, dst in ((s, s_b), (k, k_b)):
                bps = ps2.tile([128, 128], F32, tag="ps2")
                nc.tensor.matmul(bps[:, :1], lhsT=ones_r, rhs=src,
                                 start=True, stop=True)
                nc.scalar.copy(dst, bps[:, :1])

        def col_to_row(col, tag):
            rp = ps2.tile([128, 128], F32, tag="ps2")
            nc.tensor.matmul(rp[:1, :], lhsT=col, rhs=ident,
                             start=True, stop=True)
            row = scp.tile([1, 128], F32, tag=tag, bufs=1)
            nc.scalar.copy(row, rp[:1, :])
            return row

        # ---- x prep: chunks of XCH token tiles ----
        n1T_aps = {}

        def x_prep_chunk(mc):
            m0 = mc * XCH
            nt = min(XCH, TT - m0)
            x_t = xp.tile([128, XCH, D], F32, tag="x")
            nc.sync.dma_start(
                x_t[:, :nt, :],
                x_d[m0 * 128:(m0 + nt) * 128, :].rearrange(
                    "(t p) d -> p t d", p=128),
            )
            nc.vector.tensor_reduce(
                amax1_a[:, m0:m0 + nt], x_t[:, :nt, :], axis=AX.X, op=OP.max,
                apply_absolute_value=True,
            )
            for t in range(nt):
                for j in range(D // 512):
                    nc.scalar.activation(
                        dead_g[:, :512].bitcast(BF16) if False else dead_g[:, :512],
                        x_t[:, t, j * 512:(j + 1) * 512], AF.Square,
                        accum_out=sx_a[:, m0 + t, j:j + 1],
                    )
            nc.vector.tensor_reduce(
                ssq1_a[:, m0:m0 + nt], sx_a[:, m0:m0 + nt, :],
                axis=AX.X, op=OP.add)
            rec = scp.tile([128, XCH], F32, tag="rec1")
            nc.vector.reciprocal(rec[:, :nt], amax1_a[:, m0:m0 + nt])
            nc.vector.tensor_scalar_mul(c1_a[:, m0:m0 + nt], rec[:, :nt], 127.0)
            # alpha1 for the whole chunk, batched: al1 = amax1*kb1*rsqrt(u)
            u = scp.tile([128, XCH], F32, tag="al1_u")
            nc.vector.tensor_scalar(u[:, :nt], ssq1_a[:, m0:m0 + nt], 1.0 / D,
                                    EPS, op0=OP.mult, op1=OP.add)
            y = rsqrt_col(u[:, :nt], "al1")
            p = scp.tile([128, XCH], F32, tag="al1_p")
            nc.vector.tensor_scalar(p[:, :nt], amax1_a[:, m0:m0 + nt],
                                    k1_b, None, op0=OP.mult)
            nc.vector.tensor_tensor(al1_a[:, m0:m0 + nt], p[:, :nt], y,
                                    OP.mult)
            xr = nc.gpsimd if XR_POOL else nc.vector
            for t in range(nt):
                m = m0 + t
                # round step 1 (fma: x*c1 + RND), in place
                xr.tensor_scalar(x_t[:, t, :], x_t[:, t, :],
                                 c1_a[:, m:m + 1], RND,
                                 op0=OP.mult, op1=OP.add)
                n1 = n1p.tile([128, D], BF16, tag="n1")
                xr.tensor_scalar(n1, x_t[:, t, :], -RND, None,
                                 op0=OP.add)
                n1T = n1tp.tile([128, KD, 128], BF16, tag="n1T")
                nc.sync.dma_start_transpose(n1T, n1)
                n1T_aps[m] = n1T

        # ---- weight chunk pipelines ----
        def w_abs_chunk(src_ap, part_col, eng):
            """DMA f32 chunk + abs-sum partial into part_col [128,1].
            eng: 'dve' (tensor_reduce) or 'act' (Abs + accum_out)."""
            shp = src_ap.shape
            wid = int(np.prod(shp[1:]))
            wf = wstage.tile([128, wid], F32, tag="wf", bufs=3)
            dst = (wf.rearrange("p (a b) -> p a b", a=shp[1])
                   if len(shp) == 3 else wf)
            nc.sync.dma_start(dst, src_ap)
            if eng == "act":
                cw = F // 4
                nchunk = (wid + cw - 1) // cw
                pcs = scp.tile([128, nchunk], F32, tag="wabsp")
                for ci in range(nchunk):
                    nc.scalar.activation(dead_g, wf[:, ci * cw:(ci + 1) * cw],
                                         AF.Abs, accum_out=pcs[:, ci:ci + 1])
                nc.vector.tensor_reduce(part_col, pcs, axis=AX.X, op=OP.add)
            else:
                nc.vector.tensor_reduce(part_col, wf, axis=AX.X, op=OP.add,
                                        apply_absolute_value=True)

        def cast_act(dst_ap, wtr):
            nc.scalar.copy(dst_ap, wtr)

        def cast_pool(dst_ap, wtr):
            nc.gpsimd.tensor_copy(dst_ap, wtr)

        def tern_chunk(src_ap, s_b, dst_slices, e_clip1=None, e_clip2=None,
                       cast_engs=None):
            """Load f32 chunk [128, n*1024], ternarize with the single-
            rounding RND trick (f32 ulp at 1.5*2^23 is exactly 1.0):
            min -> max+RND -> -RND (bf16 {-1,0,1}) -> bf16 transpose ->
            fp8 cast (plain copy)."""
            shp = src_ap.shape
            wid = int(np.prod(shp[1:]))
            wf = wstage.tile([128, wid], F32, tag="wf", bufs=3)
            dst = (wf.rearrange("p (a b) -> p a b", a=shp[1])
                   if len(shp) == 3 else wf)
            nc.sync.dma_start(dst, src_ap)
            (e_clip1 or nc.gpsimd).tensor_scalar(wf, wf, s_b, 1.0,
                                                 op0=OP.mult, op1=OP.min)
            (e_clip2 or nc.vector).tensor_scalar(wf, wf, -1.0, RND,
                                                 op0=OP.max, op1=OP.add)
            wq = wstage.tile([128, wid], BF16, tag="wq", bufs=2)
            nc.vector.tensor_scalar(wq, wf, -RND, None, op0=OP.add)
            for i, dst_ap in enumerate(dst_slices):
                wtr = wstage.tile([128, 8, 128], BF16, tag="wtr", bufs=2)
                nc.sync.dma_start_transpose(wtr, wq[:, i * 1024:(i + 1) * 1024])
                ce = (cast_engs[i % len(cast_engs)] if cast_engs
                      else cast_act)
                ce(dst_ap, wtr)

        def tern_chunk_pe(src_ap, s_b, c):
            """w1 pass-2 chunk via PE-mode transposes (PE is idle during
            startup; saves the 0.5MB/chunk bf16 DMA transpose): load f32,
            ternarize to bf16 (+192 RNE trick), 8 PE transposes into one
            bf16 PSUM bank per 128-row group, one DVE cast (bias -192) to
            fp8 w1qT."""
            shp = src_ap.shape
            wid = int(np.prod(shp[1:]))
            wf = wstage.tile([128, wid], F32, tag="wf", bufs=3)
            dst = (wf.rearrange("p (a b) -> p a b", a=shp[1])
                   if len(shp) == 3 else wf)
            nc.sync.dma_start(dst, src_ap)
            nc.gpsimd.tensor_scalar(wf, wf, s_b, 1.0,
                                    op0=OP.mult, op1=OP.min)
            nc.vector.tensor_scalar(wf, wf, -1.0, RND,
                                    op0=OP.max, op1=OP.add)
            wq = wstage.tile([128, wid], BF16, tag="wq", bufs=2)
            nc.vector.tensor_scalar(wq, wf, -RND, None, op0=OP.add)
            for t in range(WR1):
                ptr = ps1.tile([128, KD, 128], BF16, tag="ps1",
                               name=f"w1tr_{c}_{t}")
                for k in range(KD):
                    nc.tensor.transpose(
                        ptr[:, k, :], wq[:, t * D + k * 128:t * D + (k + 1) * 128],
                        ident_bf)
                fs = slice((c * WR1 + t) * 128, (c * WR1 + t + 1) * 128)
                nc.vector.tensor_copy(w1qT[:, :, fs], ptr)

        def w1_src(c):
            return w1_d[c * 128 * WR1:(c + 1) * 128 * WR1, :].rearrange(
                "(t p) d -> p t d", p=128)

        def w1_dsts(c):
            return [w1qT[:, :, (c * WR1 + t) * 128:(c * WR1 + t + 1) * 128]
                    for t in range(WR1)]

        def w2_src(c):
            r, hh = divmod(c, NW2H)
            return w2_d[r * 128:(r + 1) * 128, hh * W2CW:(hh + 1) * W2CW]

        def w2_dsts(c):
            r, hh = divmod(c, NW2H)
            nsub = W2CW // 1024
            return [w2qT[:, (hh * nsub + t) * 8:(hh * nsub + t + 1) * 8,
                         r * 128:(r + 1) * 128] for t in range(nsub)]

        # ---- per-token-tile alpha2: al2 = amax2*kb2*rsqrt(mean g^2 + eps)
        al2_done = set()

        def alpha2_smalls(m):
            sg = scp.tile([128, 1], F32, tag="al2_sg")
            sgb = scp.tile([128, 1], F32, tag="al2_sgb")
            nc.vector.tensor_tensor(sg, ssg_a[:, m, 0:1], ssg_a[:, m, 1:2],
                                    OP.add)
            nc.vector.tensor_tensor(sgb, ssg2_a[:, m, 0:1],
                                    ssg2_a[:, m, 1:2], OP.add)
            nc.vector.tensor_tensor(sg, sg, sgb, OP.add)
            u = scp.tile([128, 1], F32, tag="al2_u")
            nc.vector.tensor_scalar(u, sg, 1.0 / F, EPS,
                                    op0=OP.mult, op1=OP.add)
            y = rsqrt_col(u, "al2")
            p = scp.tile([128, 1], F32, tag="al2_p")
            nc.vector.tensor_scalar(p, amax2_a[:, m:m + 1], k2_b, None,
                                    op0=OP.mult)
            nc.vector.tensor_tensor(al2_a[:, m:m + 1], p, y, OP.mult)
            al2_done.add(m)

        def mm1_side(m):
            if has_b1:
                ia1 = scp.tile([128, 1], F32, tag="ia1")
                nc.vector.reciprocal(ia1, al1_a[:, m:m + 1])
                ia1_row = col_to_row(ia1, "ia1r")
            g_t = gp.tile([128, F], F32, tag="g")
            for fb in range(NF1 // FB):
                p1s = [ps1.tile([128, 512], F32, tag="ps1", name=f"p1_{fb}_{i}")
                       for i in range(FB)]
                for k in range(KD):
                    for fi in range(FB):
                        f = fb * FB + fi
                        nc.tensor.matmul(
                            p1s[fi], lhsT=n1T_aps[m][:, k, :],
                            rhs=w1qT[:, k, f * 512:(f + 1) * 512],
                            start=(k == 0), stop=(k == KD - 1 and not has_b1),
                        )
                for fi in range(FB):
                    f = fb * FB + fi
                    p1 = p1s[fi]
                    if has_b1:
                        bt1 = scp.tile([1, 512], F32, tag="bias", bufs=1)
                        nc.sync.dma_start(bt1, b1_d[:, f * 512:(f + 1) * 512])
                        nc.tensor.matmul(p1, lhsT=ia1_row, rhs=bt1,
                                         start=False, stop=True)
                    fs = slice(f * 512, (f + 1) * 512)
                    if not has_b1 and not SKIP_GMAX:
                        # raw-h max straight off PSUM, in parallel with gelu:
                        # max|g| = gelu(max h) (gelu monotone, max h >> 0.35)
                        nc.vector.tensor_reduce(
                            gmax_a[:, m, f:f + 1], p1, axis=AX.X, op=OP.max)
                    nc.scalar.activation(g_t[:, fs], p1, AF.Gelu,
                                         scale=al1_a[:, m:m + 1])
                    if has_b1:
                        nc.vector.tensor_reduce(
                            gmax_a[:, m, f:f + 1], g_t[:, fs], axis=AX.X,
                            op=OP.max, apply_absolute_value=True)
            # Sum(g^2) on ACT (Square + accum), reading g_t pre-round
            for hh in range(0 if SKIP_SQ else 2):
                gs = slice(hh * (F // 2), hh * (F // 2) + F // 4)
                gs2 = slice(hh * (F // 2) + F // 4, (hh + 1) * (F // 2))
                nc.scalar.activation(dead_g, g_t[:, gs], AF.Square,
                                     accum_out=ssg_a[:, m, hh:hh + 1])
                nc.scalar.activation(dead_g, g_t[:, gs2], AF.Square,
                                     accum_out=ssg2_a[:, m, hh:hh + 1])
            if SKIP_GMAX:
                nc.gpsimd.memset(amax2_a[:, m:m + 1], 100.0)
            elif not has_b1:
                pmax = scp.tile([128, 1], F32, tag="pmax")
                nc.vector.tensor_reduce(
                    pmax, gmax_a[:, m, :], axis=AX.X, op=OP.max)
                nc.scalar.activation(amax2_a[:, m:m + 1], pmax, AF.Gelu,
                                     scale=al1_a[:, m:m + 1])
            else:
                nc.vector.tensor_reduce(
                    amax2_a[:, m:m + 1], gmax_a[:, m, :], axis=AX.X, op=OP.max)
            rec2 = scp.tile([128, 1], F32, tag="rec2")
            nc.vector.reciprocal(rec2, amax2_a[:, m:m + 1])
            nc.vector.tensor_scalar_mul(c2_a[:, m:m + 1], rec2, 127.0)
            # round trick, split across DVE and ACT by QSPLIT fraction
            FH = int(round(F * QSPLIT / 512)) * 512
            if SKIP_RND:
                FH = 0
            n2 = n2p.tile([128, F], BF16, tag="n2")
            if SKIP_RND:
                nc.gpsimd.memset(n2, 1.0)
            if FH:
                nc.vector.tensor_scalar(g_t[:, :FH], g_t[:, :FH],
                                        c2_a[:, m:m + 1], RND,
                                        op0=OP.mult, op1=OP.add)
                nc.vector.tensor_scalar(n2[:, :FH], g_t[:, :FH], -RND, None,
                                        op0=OP.add)
            if FH < F and not SKIP_RND:
                nc.scalar.activation(g_t[:, FH:], g_t[:, FH:], AF.Copy,
                                     bias=RND, scale=c2_a[:, m:m + 1])
                nc.scalar.activation(n2[:, FH:], g_t[:, FH:], AF.Copy,
                                     bias=-RND)
            nc.sync.dma_start(n2_d[m * 128:(m + 1) * 128, :], n2)

        # ---- phase B ----
        n2T_aps = {}

        def n2t_load(j):
            n2T = n2tp.tile([128, KF, 128], BF16, tag="n2T")
            nc.sync.dma_start_transpose(n2T, n2_d[j * 128:(j + 1) * 128, :])
            n2T_aps[j] = n2T

        def mm2_side(j):
            n2T = n2T_aps.pop(j)
            if has_b2:
                ia2 = scp.tile([128, 1], F32, tag="ia2")
                nc.vector.reciprocal(ia2, al2_a[:, j:j + 1])
                ia2_row = col_to_row(ia2, "ia2r")
            o_t = outp.tile([128, D], F32, tag="o")
            p2s = [ps2.tile([128, 512], F32, tag="ps2", name=f"p2_{i}")
                   for i in range(ND2)]
            for k2 in range(KF):
                for d in range(ND2):
                    nc.tensor.matmul(
                        p2s[d], lhsT=n2T[:, k2, :],
                        rhs=w2qT[:, k2, d * 512:(d + 1) * 512],
                        start=(k2 == 0), stop=(k2 == KF - 1 and not has_b2),
                    )
            for d in range(ND2):
                p2 = p2s[d]
                if has_b2:
                    bt2 = scp.tile([1, 512], F32, tag="bias", bufs=1)
                    nc.sync.dma_start(bt2, b2_d[:, d * 512:(d + 1) * 512])
                    nc.tensor.matmul(p2, lhsT=ia2_row, rhs=bt2,
                                     start=False, stop=True)
                ds_ = slice(d * 512, (d + 1) * 512)
                nc.scalar.activation(o_t[:, ds_], p2, AF.Copy,
                                     scale=al2_a[:, j:j + 1])
            nc.sync.dma_start(out_d[j * 128:(j + 1) * 128, :], o_t)

        # ================= emission =================
        rep_ctx = tc.For_i(0, REPEAT, 1) if REPEAT > 1 else None
        if rep_ctx is not None:
            ctx.enter_context(rep_ctx)
        # w1 pass 1 first (critical path to first matmul), reduce split DVE/ACT
        for c in range(NW1C):
            w_abs_chunk(w1_src(c), w1part[:, c:c + 1],
                        eng=("dve" if WABS_DVE else
                             ("act" if c % 2 else "dve")))
        finalize_mean(w1part, F * D, s1_b, k1_b)
        x_prep_chunk(0)
        if NXC > 1:
            x_prep_chunk(1)
        # w1 pass 2 via PE transposes (PE idle at startup, saves 8MB DMA)
        for c in range(NW1C):
            tern_chunk_pe(w1_src(c), s1_b, c)

        # phase A with interleaved w2 prep, phase B interleaved at LAG
        LAG = min(LAG_P, TT)
        LD = min(2, LAG)  # n2T load lead (iterations before its mm2)
        W2SPREAD = max(W2SP if W2SP is not None else LAG - 1, 1)
        w2_per = max(1, (2 * NW2C + W2SPREAD - 1) // W2SPREAD)
        w2p1_next = 0
        w2p2_next = 0
        for it in range(TT + LAG):
            jl = it - LAG + LD
            if 0 <= jl < TT and jl <= it and not SKIP_B:
                n2t_load(jl)
            if it < TT:
                mc = it // XCH + 2
                if it % XCH == 0 and mc < NXC:
                    x_prep_chunk(mc)
                mm1_side(it)
            j = it - LAG
            if 0 <= j < TT and not SKIP_B:
                if j not in al2_done:
                    alpha2_smalls(j)
                mm2_side(j)
            # w2 prep bulk ops emitted AFTER the iteration's critical work
            if W2START <= it < TT and not SKIP_W2:
                w2_target = min(2 * NW2C,
                                ((it - W2START + 1) * 2 * NW2C + W2SPREAD - 1)
                                // W2SPREAD)
                if it >= TT - 2:
                    w2_target = 2 * NW2C
                while w2p1_next + w2p2_next < w2_target:
                    if w2p1_next < NW2C:
                        c = w2p1_next
                        w_abs_chunk(w2_src(c), w2part[:, c:c + 1],
                                    eng=("dve" if WABS_DVE else
                                         ("act" if c % 2 else "dve")))

                        w2p1_next += 1
                        if w2p1_next == NW2C:
                            finalize_mean(w2part, D * F, s2_b, k2_b)
                    elif w2p2_next < NW2C:
                        tern_chunk(w2_src(w2p2_next), s2_b,
                                   w2_dsts(w2p2_next),
                                   cast_engs=[cast_act, cast_pool])
                        w2p2_next += 1

    nc.compile()
    return nc


_NC_CACHE = {}


def _get_nc(T_core, D, F, has_b1, has_b2):
    key = (T_core, D, F, has_b1, has_b2)
    if key not in _NC_CACHE:
        _NC_CACHE[key] = build_bitffn(T_core, D, F, has_b1, has_b2)
    return _NC_CACHE[key]


def kernel(x, w1, b1, w2, b2):
    B, S, D = x.shape
    Fdim = w1.shape[0]
    T = B * S
    T_core = T // NCORES
    has_b1 = bool(np.any(b1))
    has_b2 = bool(np.any(b2))

    nc = _get_nc(T_core, D, Fdim, has_b1, has_b2)

    xf = np.ascontiguousarray(x.reshape(T, D).astype(np.float32))
    w1c = np.ascontiguousarray(w1.astype(np.float32))
    w2c = np.ascontiguousarray(w2.astype(np.float32))
    b1c = np.ascontiguousarray(b1.reshape(1, Fdim).astype(np.float32))
    b2c = np.ascontiguousarray(b2.reshape(1, D).astype(np.float32))

    in_maps = [
        {
            "x": xf[i * T_core:(i + 1) * T_core],
            "w1": w1c,
            "b1": b1c,
            "w2": w2c,
            "b2": b2c,
        }
        for i in range(NCORES)
    ]
    res = bass_utils.run_bass_kernel_spmd(
        nc, in_maps=in_maps, core_ids=list(range(NCORES))
    )
    out = np.concatenate([res.results[i]["out"] for i in range(NCORES)], axis=0)
    return out.reshape(B, S, D).astype(np.float32)
